# revision 3
# baseline (speedup 1.0000x reference)
# GQA attention (B=2, T=2048, DM=2048, H=16, KV=4, D=128) + RoPE + causal mask
# on 8 TRN2 NeuronCores.
#
# Sharding: rank r = (batch b = r//4, kv-group g = r%4).  Each rank computes
# q-heads 4g..4g+3 and kv-head g for batch b (full sequence), does the SDPA
# head-sharded, then AllGathers attention outputs within each 4-rank batch
# group (chunked, overlapped with attention of later heads).  The o_proj is
# column-sharded: each rank multiplies the gathered O^T by its 512-column
# slice of Wo, accumulating in SBUF as AllGather chunks arrive, and returns
# o^T [512, 2048] bf16; the host transposes/casts and concatenates.
#
# All matmuls run in bf16 (f32 PSUM accumulation); softmax runs without
# max-subtraction (scores are bounded ~|6|); exp on ScalarE over wide
# [128,1024] PSUM tiles; denominators via bf16 running sums + a ones-[128x128]
# matmul that reduces over the k-partition axis and broadcasts; reciprocal as
# exp(-ln(x)) so ScalarE stays on one activation-table set the whole kernel.

import os
import sys

import numpy as np

for _p in ("/opt/trn_rl_repo", "/root/.axon_site/_ro/trn_rl_repo"):
    if os.path.isdir(_p) and _p not in sys.path:
        sys.path.insert(0, _p)

import ml_dtypes

import concourse.bass as bass
import concourse.mybir as mybir
import concourse.tile as tile
import concourse.masks as masks
from concourse import bacc
from concourse.bass_utils import run_bass_kernel_spmd

BF16 = ml_dtypes.bfloat16

B, T, DM = 2, 2048, 2048
H, KV, D = 16, 4, 128
NH = H // KV  # 4 local q heads per rank
P = 128
NCORES = 8
NT = T // 512  # 4 free-dim chunks of 512
NDC = DM // P  # 16 contraction chunks
SCALE = float(D) ** -0.5
ROPE_BASE = 10000.0

_bf = mybir.dt.bfloat16
_f32 = mybir.dt.float32
_EXP = mybir.ActivationFunctionType.Exp
_LOG = mybir.ActivationFunctionType.Ln


def _host_tables():
    inv = 1.0 / (ROPE_BASE ** (np.arange(0, D, 2, dtype=np.float32) / D))
    t = np.arange(T, dtype=np.float32)
    fr = np.outer(t, inv)  # [T, 64]
    emb = np.concatenate([fr, fr], axis=-1)  # [T, D]
    cosT = np.ascontiguousarray(np.cos(emb).T).astype(BF16)  # [D, T]
    sinT = np.sin(emb).T
    sinTs = np.concatenate([-sinT[:64], sinT[64:]], axis=0)
    sinTs = np.ascontiguousarray(sinTs).astype(BF16)
    i = np.arange(P)[:, None]
    j = np.arange(P)[None, :]
    tri = (i <= j).astype(BF16)  # [128, 128] upper-triangular keep-mask
    return cosT, sinTs, tri


def _kernel_body(tc, nc, xT, wq, wk, wv, wo, cosT, sinTs, tri, out):
    with (
        tc.tile_pool(name="cpool", bufs=1) as cpool,
        tc.tile_pool(name="qkvp", bufs=1) as qkvp,
        tc.tile_pool(name="wop", bufs=1) as wop,
        tc.tile_pool(name="dram", bufs=1, space="DRAM") as dram,
    ):
        # ---- persistent SBUF tensors ----
        tri_sb = cpool.tile([P, P], _bf, name="tri")
        ones_sb = cpool.tile([P, P], _bf, name="ones")

        qT = qkvp.tile([P, NH, T], _bf, name="qT")
        kT = qkvp.tile([P, T], _bf, name="kT")
        v_sb = [qkvp.tile([P, D], _bf, name=f"v{tt}") for tt in range(NDC)]

        wo_sb = wop.tile([P, H, 512], _bf, name="wo_sb")
        oacc = [
            [wop.tile([P, 512], _bf, name=f"oacc{mt}_{tcn}") for tcn in range(NT)]
            for mt in range(4)
        ]

        # AllGather chunk groups per head: h3 runs big->small so the last
        # (fully exposed) attention+collective+o_proj chain is the smallest
        groups = {h: ([0, 1], [2, 3]) for h in range(NH - 1)}
        groups[NH - 1] = ([2, 3], [1], [0])
        ag_in = dram.tile([NH, NT, P, 512], _bf, name="ag_in")
        ag_out = {
            (h, gi): dram.tile([KV, len(g), P, 512], _bf, name=f"ag_out{h}_{gi}")
            for h in range(NH)
            for gi, g in enumerate(groups[h])
        }
        warm_in = dram.tile([P, 8], _bf, name="warm_in")
        warm_out = dram.tile([KV, P, 8], _bf, name="warm_out")

        # ---- QKV projections (+ fused RoPE for q, k) ----
        # qT/kT layout [d, t]: out = W_chunk.T @ xT_chunk accumulated over DM.
        # Contraction (dc) is the OUTER loop with 4 t-accumulators so the PE
        # consumes x chunks as their DMAs land, and each stationary weight
        # load serves 4 matmuls.
        with (
            tc.tile_pool(name="xpool", bufs=1) as xpool,
            tc.tile_pool(name="wpool", bufs=1) as wpool,
            tc.tile_pool(name="psA", bufs=1, space="PSUM") as psA,
            tc.tile_pool(name="rope", bufs=2) as rp,
        ):
            x_sb = [
                xpool.tile([P, T], _bf, tag=f"x{dc}", name=f"x{dc}")
                for dc in range(NDC)
            ]
            wq_sb = wpool.tile([P, NDC, NH * D], _bf, name="wq_sb")
            wk_sb = wpool.tile([P, NDC, D], _bf, name="wk_sb")
            wv_sb = wpool.tile([P, NDC, D], _bf, name="wv_sb")
            cos_sb = wpool.tile([P, T], _bf, name="cos_sb")
            sin_sb = wpool.tile([P, T], _bf, name="sin_sb")
            ident = wpool.tile([P, P], _bf, name="ident")
            scr = wpool.tile([P, 8], _f32, name="scr")

            # x streams on the sync HWDGE ring; weights go on the scalar
            # (ACT) HWDGE ring so the two flows don't head-of-line block
            # each other at startup
            for dc in range(NDC):
                nc.sync.dma_start(x_sb[dc][:], xT[dc * P : (dc + 1) * P, :])
            nc.scalar.dma_start(wk_sb[:], wk)
            nc.scalar.dma_start(wv_sb[:], wv)
            nc.scalar.dma_start(cos_sb[:], cosT)
            nc.scalar.dma_start(sin_sb[:], sinTs)
            nc.scalar.dma_start(tri_sb[:], tri)
            nc.scalar.dma_start(wq_sb[:], wq)
            nc.scalar.dma_start(wo_sb[:], wo)
            nc.vector.memset(ones_sb[:], 1.0)
            masks.make_identity(nc, ident[:])

            # preload the one activation-table set (exp+ln) off the critical
            # path, and warm up the collectives path, both under the x load
            nc.scalar.activation(scr[:], ones_sb[:, :8], _EXP)
            nc.scalar.activation(scr[:], scr[:], _LOG)
            nc.scalar.dma_start(warm_in[:], ones_sb[:, :8])
            nc.gpsimd.collective_compute(
                "AllGather",
                mybir.AluOpType.bypass,
                replica_groups=[[0, 1, 2, 3], [4, 5, 6, 7]],
                ins=[warm_in.opt()],
                outs=[warm_out.opt()],
            )

            def rope(ps, tcn, dst):
                # RoPE: rot = src*cos + swap(src)*sin_signed, all in bf16 so
                # the DVE tensor_tensor ops run in 2x mode.  The halves-swap
                # is two SBUF->SBUF DMAs on the scalar ring.
                ts = slice(tcn * 512, (tcn + 1) * 512)
                src = rp.tile([P, 512], _bf, tag="rsrc", name="rsrc")
                nc.scalar.copy(src[:], ps[:])
                swp = rp.tile([P, 512], _bf, tag="rswp", name="rswp")
                nc.scalar.dma_start(swp[0:64, :], src[64:128, :])
                nc.scalar.dma_start(swp[64:128, :], src[0:64, :])
                nc.vector.tensor_mul(src[:], src[:], cos_sb[:, ts])
                nc.vector.tensor_mul(swp[:], swp[:], sin_sb[:, ts])
                nc.vector.tensor_add(dst, src[:], swp[:])

            def wproj(lhs_of_dc, consume):
                pss = [
                    psA.tile([P, 512], _f32, tag=f"proj{t}", name=f"proj{t}")
                    for t in range(NT)
                ]
                for dc in range(NDC):
                    lhs = lhs_of_dc(dc)
                    for tcn in range(NT):
                        nc.tensor.matmul(
                            pss[tcn][:],
                            lhs,
                            x_sb[dc][:, tcn * 512 : (tcn + 1) * 512],
                            start=(dc == 0),
                            stop=(dc == NDC - 1),
                        )
                for tcn in range(NT):
                    consume(tcn, pss[tcn])

            # k first, then v, so attention can begin as soon as q heads land
            wproj(
                lambda dc: wk_sb[:, dc, :],
                lambda tcn, ps: rope(ps, tcn, kT[:, tcn * 512 : (tcn + 1) * 512]),
            )

            # v: v^T = Wv^T @ x^T, then PE-transpose [128,128] blocks into the
            # natural [t, d] tiles AV needs
            def consume_v(tcn, ps):
                vT_sb = rp.tile([P, 512], _bf, tag="vTsb", name="vTsb")
                nc.scalar.copy(vT_sb[:], ps[:])
                for sub in range(4):
                    tt = tcn * 4 + sub
                    ptr = psA.tile([P, P], _bf, tag="vtr", name="vtr", bufs=2)
                    nc.tensor.transpose(
                        ptr[:], vT_sb[:, sub * P : (sub + 1) * P], ident[:]
                    )
                    nc.vector.tensor_copy(v_sb[tt][:], ptr[:])

            wproj(lambda dc: wv_sb[:, dc, :], consume_v)
            for j in range(NH):
                wproj(
                    lambda dc, j=j: wq_sb[:, dc, j * P : (j + 1) * P],
                    lambda tcn, ps, j=j: rope(
                        ps, tcn, qT[:, j, tcn * 512 : (tcn + 1) * 512]
                    ),
                )

        # ---- attention + chunked AllGather + pipelined o_proj ----
        # scores^T [k, q] per head with causal block skip; the two q-chunks of
        # a group share a wide [128,1024] PSUM tile so exp and the running
        # denominator sum are single wide ops per kt.
        with (
            tc.tile_pool(name="psS", bufs=2, space="PSUM") as psS,
            tc.tile_pool(name="psAV", bufs=1, space="PSUM") as psAV,
            tc.tile_pool(name="psO", bufs=1, space="PSUM") as psO,
            tc.tile_pool(name="att", bufs=3) as ap_,
            tc.tile_pool(name="expp", bufs=4) as expp,
            tc.tile_pool(name="accp", bufs=2) as accp,
            tc.tile_pool(name="agsb", bufs=2) as agsb,
        ):
            def issue_loads(ph, pgi, pg):
                sb = {}
                for i in range(KV):
                    for l, qc in enumerate(pg):
                        t_ = agsb.tile(
                            [P, 512], _bf, tag=f"ag{i}_{qc % 2}", name=f"ag{ph}{i}{qc}"
                        )
                        nc.sync.dma_start(t_[:], ag_out[ph, pgi][i, l])
                        sb[i, qc] = t_
                return sb

            def consume_mms(ph, pg, sb):
                # o_proj contribution of global heads {4i+ph} for this chunk's
                # t-columns, accumulated into SBUF (bf16)
                for mt in range(4):
                    pos = {
                        qc: psO.tile(
                            [P, 512], _f32, tag=f"pos{qc % 2}", name=f"pos{qc}"
                        )
                        for qc in pg
                    }
                    for i in range(KV):
                        lhs = wo_sb[:, 4 * i + ph, mt * P : (mt + 1) * P]
                        for qc in pg:
                            nc.tensor.matmul(
                                pos[qc][:],
                                lhs,
                                sb[i, qc][:],
                                start=(i == 0),
                                stop=(i == KV - 1),
                            )
                    for qc in pg:
                        if ph == 0:
                            nc.vector.tensor_copy(oacc[mt][qc][:], pos[qc][:])
                        else:
                            nc.vector.tensor_add(
                                oacc[mt][qc][:], oacc[mt][qc][:], pos[qc][:]
                            )
                        if ph == NH - 1:
                            nc.sync.dma_start(
                                out[
                                    mt * P : (mt + 1) * P,
                                    qc * 512 : (qc + 1) * 512,
                                ],
                                oacc[mt][qc][:],
                            )

            pending = None
            pend_sb = None
            for h in range(NH):
                for gi, g in enumerate(groups[h]):
                    ng = len(g)
                    # issue the PREVIOUS chunk's AllGather-output loads now;
                    # they complete while this chunk's attention runs
                    if pending is not None:
                        pend_sb = issue_loads(*pending)
                    acc = accp.tile(
                        [P, ng * 512], _bf, tag="acc", name=f"acc{h}{gi}"
                    )
                    avs = {
                        qc: psAV.tile(
                            [P, 512], _f32, tag=f"av{i}", name=f"av{h}_{qc}"
                        )
                        for i, qc in enumerate(g)
                    }
                    pend_av = None
                    for kt in range(4 * max(g) + 4):
                        lhs_k = kT[:, kt * P : (kt + 1) * P]
                        valid = [qc for qc in g if kt <= 4 * qc + 3]
                        ps = psS.tile([P, 1024], _f32, tag="s", name="s")
                        ex = expp.tile([P, 1024], _bf, tag="exp", name="ex")
                        exs = {}
                        for qc in valid:
                            ai = qc - g[0]
                            bound = kt // 4 == qc
                            off = 128 * (kt % 4) if bound else 0
                            lo = ai * 512 + off
                            hi = (ai + 1) * 512
                            qs = slice(qc * 512 + off, (qc + 1) * 512)
                            nc.tensor.matmul(
                                ps[:, lo:hi], lhs_k, qT[:, h, qs], start=True, stop=True
                            )
                            exs[qc] = (ex, lo, hi, off, bound)
                        lo0 = min(e[1] for e in exs.values())
                        hi0 = max(e[2] for e in exs.values())
                        nc.scalar.activation(
                            ex[:, lo0:hi0], ps[:, lo0:hi0], _EXP, scale=SCALE
                        )
                        for qc, (exw, lo, hi, off, bound) in exs.items():
                            if bound:
                                nc.vector.tensor_mul(
                                    exw[:, lo : lo + P], exw[:, lo : lo + P], tri_sb[:]
                                )
                        if kt == 0:
                            nc.vector.tensor_copy(acc[:], ex[:, : ng * 512])
                        else:
                            nc.vector.tensor_add(
                                acc[:, lo0:hi0], acc[:, lo0:hi0], ex[:, lo0:hi0]
                            )
                        # AV runs one kt behind scores so the PE never waits
                        # on ScalarE's exp round-trip
                        if pend_av is not None:
                            pkt, pexs = pend_av
                            for qc, (exw, lo, hi, off, bound) in pexs.items():
                                nc.tensor.matmul(
                                    avs[qc][:, off:],
                                    v_sb[pkt][:],
                                    exw[:, lo:hi],
                                    start=(pkt == 0),
                                    stop=(pkt == 4 * qc + 3),
                                )
                        pend_av = (kt, exs)
                    pkt, pexs = pend_av
                    for qc, (exw, lo, hi, off, bound) in pexs.items():
                        nc.tensor.matmul(
                            avs[qc][:, off:],
                            v_sb[pkt][:],
                            exw[:, lo:hi],
                            start=(pkt == 0),
                            stop=(pkt == 4 * qc + 3),
                        )
                    # denominators (k-partition sum + broadcast via ones
                    # matmul), reciprocal as exp(-ln(x)), normalize, ship out
                    for qc in g:
                        ai = qc - g[0]
                        dps = psS.tile([P, 1024], _f32, tag="s", name="sden")
                        nc.tensor.matmul(
                            dps[:, :512],
                            ones_sb[:],
                            acc[:, ai * 512 : (ai + 1) * 512],
                            start=True,
                            stop=True,
                        )
                        lnt = ap_.tile([P, 512], _f32, tag="lnt", name="lnt")
                        nc.scalar.activation(lnt[:], dps[:, :512], _LOG)
                        rec = ap_.tile([P, 512], _f32, tag="rec", name="rec")
                        nc.scalar.activation(rec[:], lnt[:], _EXP, scale=-1.0)
                        oq = ap_.tile([P, 512], _bf, tag="oq", name="oq")
                        nc.vector.tensor_mul(oq[:], avs[qc][:], rec[:])
                        nc.sync.dma_start(ag_in[h, qc][:, :], oq[:])

                    # AllGather this chunk across the 4-rank batch group
                    nc.gpsimd.collective_compute(
                        "AllGather",
                        mybir.AluOpType.bypass,
                        replica_groups=[[0, 1, 2, 3], [4, 5, 6, 7]],
                        ins=[ag_in[h, g[0] : g[0] + ng].opt()],
                        outs=[ag_out[h, gi].opt()],
                    )
                    # consume the PREVIOUS chunk's o_proj now that its
                    # AllGather has completed and its loads are in SBUF
                    if pending is not None:
                        consume_mms(pending[0], pending[2], pend_sb)
                    pending = (h, gi, g)
            pend_sb = issue_loads(*pending)
            consume_mms(pending[0], pending[2], pend_sb)


def build_nc():
    nc = bacc.Bacc(
        "TRN2", target_bir_lowering=False, debug=False, num_devices=NCORES
    )
    xT = nc.dram_tensor("xT", [DM, T], _bf, kind="ExternalInput").ap()
    wq = nc.dram_tensor("wq", [P, NDC, NH * D], _bf, kind="ExternalInput").ap()
    wk = nc.dram_tensor("wk", [P, NDC, D], _bf, kind="ExternalInput").ap()
    wv = nc.dram_tensor("wv", [P, NDC, D], _bf, kind="ExternalInput").ap()
    wo = nc.dram_tensor("wo", [P, H, 512], _bf, kind="ExternalInput").ap()
    cosT = nc.dram_tensor("cosT", [D, T], _bf, kind="ExternalInput").ap()
    sinTs = nc.dram_tensor("sinTs", [D, T], _bf, kind="ExternalInput").ap()
    tri = nc.dram_tensor("tri", [P, P], _bf, kind="ExternalInput").ap()
    out = nc.dram_tensor("out", [512, T], _bf, kind="ExternalOutput").ap()
    with tile.TileContext(nc) as tc:
        _kernel_body(tc, nc, xT, wq, wk, wv, wo, cosT, sinTs, tri, out)
    nc.finalize()
    return nc


def _chunked(w, cols):
    # [DM, cols] -> [P, NDC, cols] contiguous so the device DMA is one
    # clean 128-partition transfer
    return np.ascontiguousarray(
        w.reshape(NDC, P, cols).transpose(1, 0, 2)
    ).astype(BF16)


def make_in_maps(x, Wq, Wk, Wv, Wo):
    cosT, sinTs, tri = _host_tables()
    xTb = [np.ascontiguousarray(x[b].T).astype(BF16) for b in range(B)]
    wg = []
    for g in range(KV):
        wg.append(
            {
                "wq": _chunked(Wq[:, g * NH * D : (g + 1) * NH * D], NH * D),
                "wk": _chunked(Wk[:, g * D : (g + 1) * D], D),
                "wv": _chunked(Wv[:, g * D : (g + 1) * D], D),
                "wo": np.ascontiguousarray(
                    Wo[:, g * 512 : (g + 1) * 512].reshape(H, P, 512).transpose(1, 0, 2)
                ).astype(BF16),
            }
        )
    in_maps = []
    for r in range(NCORES):
        b, g = divmod(r, KV)
        m = {"xT": xTb[b], "cosT": cosT, "sinTs": sinTs, "tri": tri}
        m.update(wg[g])
        in_maps.append(m)
    return in_maps


def assemble(results):
    out = np.empty((B, T, DM), np.float32)
    for r in range(NCORES):
        b, g = divmod(r, KV)
        out[b, :, g * 512 : (g + 1) * 512] = results[r]["out"].T.astype(np.float32)
    return out


_NC_CACHE = {}


def get_nc():
    if "nc" not in _NC_CACHE:
        _NC_CACHE["nc"] = build_nc()
    return _NC_CACHE["nc"]


def run(x, Wq, Wk, Wv, Wo, trace=False, taps=False):
    nc = get_nc()
    in_maps = make_in_maps(x, Wq, Wk, Wv, Wo)
    res = run_bass_kernel_spmd(
        nc, in_maps, core_ids=list(range(NCORES)), trace=trace
    )
    return assemble(res.results), res


def kernel(x, Wq, Wk, Wv, Wo, mask=None, **_unused):
    x = np.asarray(x, dtype=np.float32)
    Wq = np.asarray(Wq, dtype=np.float32)
    Wk = np.asarray(Wk, dtype=np.float32)
    Wv = np.asarray(Wv, dtype=np.float32)
    Wo = np.asarray(Wo, dtype=np.float32)
    out, _ = run(x, Wq, Wk, Wv, Wo, trace=False)
    return out


# revision 10
# speedup vs baseline: 1.1046x; 1.1046x over previous
# GQA attention (B=2, T=2048, DM=2048, H=16, KV=4, D=128) + RoPE + causal mask
# on 8 TRN2 NeuronCores.
#
# Sharding: rank r = (batch b = r//4, kv-group g = r%4).  Each rank computes
# q-heads 4g..4g+3 and kv-head g for batch b (full sequence), does the SDPA
# head-sharded, then AllGathers attention outputs within each 4-rank batch
# group (chunked, overlapped with attention of later heads).  The o_proj is
# column-sharded: each rank multiplies the gathered O^T by its 512-column
# slice of Wo, accumulating in SBUF as AllGather chunks arrive, and returns
# o^T [512, 2048] bf16; the host transposes/casts and concatenates.
#
# All matmuls run in bf16 (f32 PSUM accumulation); softmax runs without
# max-subtraction (scores are bounded ~|6|); exp on ScalarE over wide
# [128,1024] PSUM tiles; denominators via bf16 running sums + a ones-[128x128]
# matmul that reduces over the k-partition axis and broadcasts; reciprocal as
# exp(-ln(x)) so ScalarE stays on one activation-table set the whole kernel.

import os
import sys

import numpy as np

for _p in ("/opt/trn_rl_repo", "/root/.axon_site/_ro/trn_rl_repo"):
    if os.path.isdir(_p) and _p not in sys.path:
        sys.path.insert(0, _p)

import ml_dtypes

import concourse.bass as bass
import concourse.mybir as mybir
import concourse.tile as tile
import concourse.masks as masks
from concourse import bacc
from concourse.bass_utils import run_bass_kernel_spmd

BF16 = ml_dtypes.bfloat16

B, T, DM = 2, 2048, 2048
H, KV, D = 16, 4, 128
NH = H // KV  # 4 local q heads per rank
P = 128
NCORES = 8
NT = T // 512  # 4 free-dim chunks of 512
NDC = DM // P  # 16 contraction chunks
SCALE = float(D) ** -0.5
ROPE_BASE = 10000.0

_bf = mybir.dt.bfloat16
_f32 = mybir.dt.float32
_EXP = mybir.ActivationFunctionType.Exp



def _host_tables():
    inv = 1.0 / (ROPE_BASE ** (np.arange(0, D, 2, dtype=np.float32) / D))
    t = np.arange(T, dtype=np.float32)
    fr = np.outer(t, inv)  # [T, 64]
    emb = np.concatenate([fr, fr], axis=-1)  # [T, D]
    cosT = np.ascontiguousarray(np.cos(emb).T).astype(BF16)  # [D, T]
    sinT = np.sin(emb).T
    sinTs = np.concatenate([-sinT[:64], sinT[64:]], axis=0)
    sinTs = np.ascontiguousarray(sinTs).astype(BF16)
    i = np.arange(P)[:, None]
    j = np.arange(P)[None, :]
    tri = (i <= j).astype(BF16)  # [128, 128] upper-triangular keep-mask
    return cosT, sinTs, tri


def _kernel_body(tc, nc, xT, wq, wk, wv, wo, cosT, sinTs, tri, out):
    with (
        tc.tile_pool(name="cpool", bufs=1) as cpool,
        tc.tile_pool(name="qkvp", bufs=1) as qkvp,
        tc.tile_pool(name="wop", bufs=1) as wop,
        tc.tile_pool(name="dram", bufs=1, space="DRAM") as dram,
    ):
        # ---- persistent SBUF tensors ----
        tri_sb = cpool.tile([P, P], _bf, name="tri")
        ones_sb = cpool.tile([P, P], _bf, name="ones")

        qT = qkvp.tile([P, NH, T], _bf, name="qT")
        kT = qkvp.tile([P, T], _bf, name="kT")
        v_sb = [qkvp.tile([P, D], _bf, name=f"v{tt}") for tt in range(NDC)]

        wo_sb = wop.tile([P, H, 512], _bf, name="wo_sb")
        oacc = [
            [wop.tile([P, 512], _bf, name=f"oacc{mt}_{tcn}") for tcn in range(NT)]
            for mt in range(4)
        ]

        # AllGather chunk groups per head: h3 runs big->small so the last
        # (fully exposed) attention+collective+o_proj chain is the smallest
        groups = {h: ([0, 1], [2, 3]) for h in range(NH - 1)}
        groups[NH - 1] = ([2, 3], [1], [0])
        ag_in = dram.tile([NH, NT, P, 512], _bf, name="ag_in")
        ag_out = {
            (h, gi): dram.tile([KV, len(g), P, 512], _bf, name=f"ag_out{h}_{gi}")
            for h in range(NH)
            for gi, g in enumerate(groups[h])
        }
        warm_in = dram.tile([P, 8], _bf, name="warm_in")
        warm_out = dram.tile([KV, P, 8], _bf, name="warm_out")

        # ---- QKV projections (+ fused RoPE for q, k) ----
        # qT/kT layout [d, t]: out = W_chunk.T @ xT_chunk accumulated over DM.
        # Contraction (dc) is the OUTER loop with 4 t-accumulators so the PE
        # consumes x chunks as their DMAs land, and each stationary weight
        # load serves 4 matmuls.
        with (
            tc.tile_pool(name="xpool", bufs=1) as xpool,
            tc.tile_pool(name="wpool", bufs=1) as wpool,
            tc.tile_pool(name="psA", bufs=1, space="PSUM") as psA,
            tc.tile_pool(name="rope", bufs=2) as rp,
        ):
            x_sb = [
                xpool.tile([P, T], _bf, tag=f"x{dc}", name=f"x{dc}")
                for dc in range(NDC)
            ]
            wq_sb = wpool.tile([P, NDC, NH * D], _bf, name="wq_sb")
            wk_sb = wpool.tile([P, NDC, D], _bf, name="wk_sb")
            wv_sb = wpool.tile([P, NDC, D], _bf, name="wv_sb")
            cos_sb = wpool.tile([P, T], _bf, name="cos_sb")
            sin_sb = wpool.tile([P, T], _bf, name="sin_sb")
            ident = wpool.tile([P, P], _bf, name="ident")
            scr = wpool.tile([P, 8], _f32, name="scr")

            # x streams on the sync HWDGE ring; weights go on the scalar
            # (ACT) HWDGE ring so the two flows don't head-of-line block
            # each other at startup
            for dc in range(NDC):
                nc.sync.dma_start(x_sb[dc][:], xT[dc * P : (dc + 1) * P, :])
            nc.scalar.dma_start(wk_sb[:], wk)
            nc.scalar.dma_start(wv_sb[:], wv)
            nc.scalar.dma_start(cos_sb[:], cosT)
            nc.scalar.dma_start(sin_sb[:], sinTs)
            nc.scalar.dma_start(tri_sb[:], tri)
            nc.scalar.dma_start(wq_sb[:], wq)
            nc.scalar.dma_start(wo_sb[:], wo)
            nc.vector.memset(ones_sb[:], 1.0)
            masks.make_identity(nc, ident[:])

            # preload the exp activation table off the critical path, and
            # warm up the collectives path, both under the x load
            nc.scalar.activation(scr[:], ones_sb[:, :8], _EXP)
            nc.scalar.dma_start(warm_in[:], ones_sb[:, :8])
            nc.gpsimd.collective_compute(
                "AllGather",
                mybir.AluOpType.bypass,
                replica_groups=[[0, 1, 2, 3], [4, 5, 6, 7]],
                ins=[warm_in.opt()],
                outs=[warm_out.opt()],
            )

            def rope(ps, tcn, dst):
                # RoPE: rot = src*cos + swap(src)*sin_signed, all in bf16 so
                # the DVE tensor_tensor ops run in 2x mode.  The halves-swap
                # is two SBUF->SBUF DMAs on the scalar ring.
                ts = slice(tcn * 512, (tcn + 1) * 512)
                src = rp.tile([P, 512], _bf, tag="rsrc", name="rsrc")
                nc.scalar.copy(src[:], ps[:])
                swp = rp.tile([P, 512], _bf, tag="rswp", name="rswp")
                nc.scalar.dma_start(swp[0:64, :], src[64:128, :])
                nc.scalar.dma_start(swp[64:128, :], src[0:64, :])
                nc.vector.tensor_mul(src[:], src[:], cos_sb[:, ts])
                nc.vector.tensor_mul(swp[:], swp[:], sin_sb[:, ts])
                nc.vector.tensor_add(dst, src[:], swp[:])

            def wproj(lhs_of_dc, consume):
                pss = [
                    psA.tile([P, 512], _f32, tag=f"proj{t}", name=f"proj{t}")
                    for t in range(NT)
                ]
                for dc in range(NDC):
                    lhs = lhs_of_dc(dc)
                    for tcn in range(NT):
                        nc.tensor.matmul(
                            pss[tcn][:],
                            lhs,
                            x_sb[dc][:, tcn * 512 : (tcn + 1) * 512],
                            start=(dc == 0),
                            stop=(dc == NDC - 1),
                        )
                for tcn in range(NT):
                    consume(tcn, pss[tcn])

            # k first, then v, so attention can begin as soon as q heads land
            wproj(
                lambda dc: wk_sb[:, dc, :],
                lambda tcn, ps: rope(ps, tcn, kT[:, tcn * 512 : (tcn + 1) * 512]),
            )

            # v: v^T = Wv^T @ x^T, then PE-transpose [128,128] blocks into the
            # natural [t, d] tiles AV needs
            def consume_v(tcn, ps):
                vT_sb = rp.tile([P, 512], _bf, tag="vTsb", name="vTsb")
                nc.scalar.copy(vT_sb[:], ps[:])
                for sub in range(4):
                    tt = tcn * 4 + sub
                    ptr = psA.tile([P, P], _bf, tag="vtr", name="vtr", bufs=2)
                    nc.tensor.transpose(
                        ptr[:], vT_sb[:, sub * P : (sub + 1) * P], ident[:]
                    )
                    nc.vector.tensor_copy(v_sb[tt][:], ptr[:])

            wproj(lambda dc: wv_sb[:, dc, :], consume_v)
            for j in range(NH):
                wproj(
                    lambda dc, j=j: wq_sb[:, dc, j * P : (j + 1) * P],
                    lambda tcn, ps, j=j: rope(
                        ps, tcn, qT[:, j, tcn * 512 : (tcn + 1) * 512]
                    ),
                )

        # ---- attention + chunked AllGather + pipelined o_proj ----
        # scores^T [k, q] per head with causal block skip; the two q-chunks of
        # a group share a wide [128,1024] PSUM tile so exp and the running
        # denominator sum are single wide ops per kt.
        with (
            tc.tile_pool(name="psS", bufs=2, space="PSUM") as psS,
            tc.tile_pool(name="psAV", bufs=1, space="PSUM") as psAV,
            tc.tile_pool(name="psO", bufs=1, space="PSUM") as psO,
            tc.tile_pool(name="att", bufs=3) as ap_,
            tc.tile_pool(name="expp", bufs=4) as expp,
            tc.tile_pool(name="accp", bufs=2) as accp,
            tc.tile_pool(name="agsb", bufs=2) as agsb,
        ):
            def consume_chunk(ph, pgi, pg):
                # o_proj contribution of global heads {4i+ph} for this chunk's
                # t-columns, accumulated into SBUF (bf16).  Called one chunk
                # late so the AllGather has completed and the loads don't
                # head-of-line block the sync DMA queue.
                sb = {}
                for i in range(KV):
                    for l, qc in enumerate(pg):
                        t_ = agsb.tile(
                            [P, 512], _bf, tag=f"ag{i}_{qc % 2}", name=f"ag{ph}{i}{qc}"
                        )
                        nc.sync.dma_start(t_[:], ag_out[ph, pgi][i, l])
                        sb[i, qc] = t_
                for mt in range(4):
                    pos = {
                        qc: psO.tile(
                            [P, 512], _f32, tag=f"pos{qc % 2}", name=f"pos{qc}"
                        )
                        for qc in pg
                    }
                    for i in range(KV):
                        lhs = wo_sb[:, 4 * i + ph, mt * P : (mt + 1) * P]
                        for qc in pg:
                            nc.tensor.matmul(
                                pos[qc][:],
                                lhs,
                                sb[i, qc][:],
                                start=(i == 0),
                                stop=(i == KV - 1),
                            )
                    for qc in pg:
                        if ph == 0:
                            nc.vector.tensor_copy(oacc[mt][qc][:], pos[qc][:])
                        else:
                            nc.vector.tensor_add(
                                oacc[mt][qc][:], oacc[mt][qc][:], pos[qc][:]
                            )
                        if ph == NH - 1:
                            nc.sync.dma_start(
                                out[
                                    mt * P : (mt + 1) * P,
                                    qc * 512 : (qc + 1) * 512,
                                ],
                                oacc[mt][qc][:],
                            )

            pending = None
            for h in range(NH):
                for gi, g in enumerate(groups[h]):
                    ng = len(g)
                    acc = accp.tile(
                        [P, ng * 512], _bf, tag="acc", name=f"acc{h}{gi}"
                    )
                    avs = {
                        qc: psAV.tile(
                            [P, 512], _f32, tag=f"av{i}", name=f"av{h}_{qc}"
                        )
                        for i, qc in enumerate(g)
                    }
                    def finish(qc, h=h):
                        # denominator (k-partition sum + broadcast via ones
                        # matmul), fast DVE reciprocal, normalize, ship out.
                        # Fired as soon as this q-chunk's last AV is issued so
                        # the chain hides under the remaining kt iterations.
                        ai = qc - g[0]
                        dps = psS.tile([P, 1024], _f32, tag="s", name="sden")
                        nc.tensor.matmul(
                            dps[:, :512],
                            ones_sb[:],
                            acc[:, ai * 512 : (ai + 1) * 512],
                            start=True,
                            stop=True,
                        )
                        rec = ap_.tile([P, 512], _f32, tag="rec", name="rec")
                        nc.vector.reciprocal_approx_fast(rec[:], dps[:, :512])
                        oq = ap_.tile([P, 512], _bf, tag="oq", name="oq")
                        nc.vector.tensor_mul(oq[:], avs[qc][:], rec[:])
                        nc.sync.dma_start(ag_in[h, qc][:, :], oq[:])

                    pend_av = None
                    for kt in range(4 * max(g) + 4):
                        lhs_k = kT[:, kt * P : (kt + 1) * P]
                        valid = [qc for qc in g if kt <= 4 * qc + 3]
                        ps = psS.tile([P, 1024], _f32, tag="s", name="s")
                        ex = expp.tile([P, 1024], _bf, tag="exp", name="ex")
                        exs = {}
                        for qc in valid:
                            ai = qc - g[0]
                            bound = kt // 4 == qc
                            off = 128 * (kt % 4) if bound else 0
                            lo = ai * 512 + off
                            hi = (ai + 1) * 512
                            qs = slice(qc * 512 + off, (qc + 1) * 512)
                            nc.tensor.matmul(
                                ps[:, lo:hi], lhs_k, qT[:, h, qs], start=True, stop=True
                            )
                            exs[qc] = (ex, lo, hi, off, bound)
                        lo0 = min(e[1] for e in exs.values())
                        hi0 = max(e[2] for e in exs.values())
                        nc.scalar.activation(
                            ex[:, lo0:hi0], ps[:, lo0:hi0], _EXP, scale=SCALE
                        )
                        for qc, (exw, lo, hi, off, bound) in exs.items():
                            if bound:
                                nc.vector.tensor_mul(
                                    exw[:, lo : lo + P], exw[:, lo : lo + P], tri_sb[:]
                                )
                        if kt == 0:
                            nc.vector.tensor_copy(acc[:], ex[:, : ng * 512])
                        else:
                            nc.vector.tensor_add(
                                acc[:, lo0:hi0], acc[:, lo0:hi0], ex[:, lo0:hi0]
                            )
                        # AV runs one kt behind scores so the PE never waits
                        # on ScalarE's exp round-trip
                        if pend_av is not None:
                            pkt, pexs = pend_av
                            for qc, (exw, lo, hi, off, bound) in pexs.items():
                                nc.tensor.matmul(
                                    avs[qc][:, off:],
                                    v_sb[pkt][:],
                                    exw[:, lo:hi],
                                    start=(pkt == 0),
                                    stop=(pkt == 4 * qc + 3),
                                )
                            for qc in pexs:
                                if pkt == 4 * qc + 3:
                                    finish(qc)
                        pend_av = (kt, exs)
                    pkt, pexs = pend_av
                    for qc, (exw, lo, hi, off, bound) in pexs.items():
                        nc.tensor.matmul(
                            avs[qc][:, off:],
                            v_sb[pkt][:],
                            exw[:, lo:hi],
                            start=(pkt == 0),
                            stop=(pkt == 4 * qc + 3),
                        )
                    for qc in pexs:
                        if pkt == 4 * qc + 3:
                            finish(qc)

                    # AllGather this chunk across the 4-rank batch group
                    nc.gpsimd.collective_compute(
                        "AllGather",
                        mybir.AluOpType.bypass,
                        replica_groups=[[0, 1, 2, 3], [4, 5, 6, 7]],
                        ins=[ag_in[h, g[0] : g[0] + ng].opt()],
                        outs=[ag_out[h, gi].opt()],
                    )
                    # consume the PREVIOUS chunk's AllGather now — by this
                    # point it has completed, so its loads don't head-of-line
                    # block the sync DMA queue
                    if pending is not None:
                        consume_chunk(*pending)
                    pending = (h, gi, g)
            consume_chunk(*pending)


def build_nc():
    nc = bacc.Bacc(
        "TRN2", target_bir_lowering=False, debug=False, num_devices=NCORES
    )
    xT = nc.dram_tensor("xT", [DM, T], _bf, kind="ExternalInput").ap()
    wq = nc.dram_tensor("wq", [P, NDC, NH * D], _bf, kind="ExternalInput").ap()
    wk = nc.dram_tensor("wk", [P, NDC, D], _bf, kind="ExternalInput").ap()
    wv = nc.dram_tensor("wv", [P, NDC, D], _bf, kind="ExternalInput").ap()
    wo = nc.dram_tensor("wo", [P, H, 512], _bf, kind="ExternalInput").ap()
    cosT = nc.dram_tensor("cosT", [D, T], _bf, kind="ExternalInput").ap()
    sinTs = nc.dram_tensor("sinTs", [D, T], _bf, kind="ExternalInput").ap()
    tri = nc.dram_tensor("tri", [P, P], _bf, kind="ExternalInput").ap()
    out = nc.dram_tensor("out", [512, T], _bf, kind="ExternalOutput").ap()
    with tile.TileContext(nc) as tc:
        _kernel_body(tc, nc, xT, wq, wk, wv, wo, cosT, sinTs, tri, out)
    nc.finalize()
    return nc


def _chunked(w, cols):
    # [DM, cols] -> [P, NDC, cols] contiguous so the device DMA is one
    # clean 128-partition transfer
    return np.ascontiguousarray(
        w.reshape(NDC, P, cols).transpose(1, 0, 2)
    ).astype(BF16)


def make_in_maps(x, Wq, Wk, Wv, Wo):
    cosT, sinTs, tri = _host_tables()
    xTb = [np.ascontiguousarray(x[b].T).astype(BF16) for b in range(B)]
    wg = []
    for g in range(KV):
        wg.append(
            {
                "wq": _chunked(Wq[:, g * NH * D : (g + 1) * NH * D], NH * D),
                "wk": _chunked(Wk[:, g * D : (g + 1) * D], D),
                "wv": _chunked(Wv[:, g * D : (g + 1) * D], D),
                "wo": np.ascontiguousarray(
                    Wo[:, g * 512 : (g + 1) * 512].reshape(H, P, 512).transpose(1, 0, 2)
                ).astype(BF16),
            }
        )
    in_maps = []
    for r in range(NCORES):
        b, g = divmod(r, KV)
        m = {"xT": xTb[b], "cosT": cosT, "sinTs": sinTs, "tri": tri}
        m.update(wg[g])
        in_maps.append(m)
    return in_maps


def assemble(results):
    out = np.empty((B, T, DM), np.float32)
    for r in range(NCORES):
        b, g = divmod(r, KV)
        out[b, :, g * 512 : (g + 1) * 512] = results[r]["out"].T.astype(np.float32)
    return out


_NC_CACHE = {}


def get_nc():
    if "nc" not in _NC_CACHE:
        _NC_CACHE["nc"] = build_nc()
    return _NC_CACHE["nc"]


def run(x, Wq, Wk, Wv, Wo, trace=False, taps=False):
    nc = get_nc()
    in_maps = make_in_maps(x, Wq, Wk, Wv, Wo)
    res = run_bass_kernel_spmd(
        nc, in_maps, core_ids=list(range(NCORES)), trace=trace
    )
    return assemble(res.results), res


def kernel(x, Wq, Wk, Wv, Wo, mask=None, **_unused):
    x = np.asarray(x, dtype=np.float32)
    Wq = np.asarray(Wq, dtype=np.float32)
    Wk = np.asarray(Wk, dtype=np.float32)
    Wv = np.asarray(Wv, dtype=np.float32)
    Wo = np.asarray(Wo, dtype=np.float32)
    out, _ = run(x, Wq, Wk, Wv, Wo, trace=False)
    return out


# revision 17
# speedup vs baseline: 1.1088x; 1.0037x over previous
# GQA attention (B=2, T=2048, DM=2048, H=16, KV=4, D=128) + RoPE + causal mask
# on 8 TRN2 NeuronCores.
#
# Sharding: rank r = (batch b = r//4, kv-group g = r%4).  Each rank computes
# q-heads 4g..4g+3 and kv-head g for batch b (full sequence), does the SDPA
# head-sharded, then AllGathers attention outputs within each 4-rank batch
# group (chunked, overlapped with attention of later heads).  The o_proj is
# column-sharded: each rank multiplies the gathered O^T by its 512-column
# slice of Wo, accumulating in SBUF as AllGather chunks arrive, and returns
# o^T [512, 2048] bf16; the host transposes/casts and concatenates.
#
# All matmuls run in bf16 (f32 PSUM accumulation); softmax runs without
# max-subtraction (scores are bounded ~|6|); exp on ScalarE over wide
# [128,1024] PSUM tiles; denominators via bf16 running sums + a ones-[128x128]
# matmul that reduces over the k-partition axis and broadcasts; reciprocal as
# exp(-ln(x)) so ScalarE stays on one activation-table set the whole kernel.

import os
import sys

import numpy as np

for _p in ("/opt/trn_rl_repo", "/root/.axon_site/_ro/trn_rl_repo"):
    if os.path.isdir(_p) and _p not in sys.path:
        sys.path.insert(0, _p)

import ml_dtypes

import concourse.bass as bass
import concourse.mybir as mybir
import concourse.tile as tile
import concourse.masks as masks
from concourse import bacc
from concourse.bass_utils import run_bass_kernel_spmd

BF16 = ml_dtypes.bfloat16

B, T, DM = 2, 2048, 2048
H, KV, D = 16, 4, 128
NH = H // KV  # 4 local q heads per rank
P = 128
NCORES = 8
NT = T // 512  # 4 free-dim chunks of 512
NDC = DM // P  # 16 contraction chunks
SCALE = float(D) ** -0.5
ROPE_BASE = 10000.0

_bf = mybir.dt.bfloat16
_f32 = mybir.dt.float32
_EXP = mybir.ActivationFunctionType.Exp



def _host_tables():
    inv = 1.0 / (ROPE_BASE ** (np.arange(0, D, 2, dtype=np.float32) / D))
    t = np.arange(T, dtype=np.float32)
    fr = np.outer(t, inv)  # [T, 64]
    emb = np.concatenate([fr, fr], axis=-1)  # [T, D]
    cosT = np.ascontiguousarray(np.cos(emb).T).astype(BF16)  # [D, T]
    sinT = np.sin(emb).T
    sinTs = np.concatenate([-sinT[:64], sinT[64:]], axis=0)
    sinTs = np.ascontiguousarray(sinTs).astype(BF16)
    i = np.arange(P)[:, None]
    j = np.arange(P)[None, :]
    tri = (i <= j).astype(BF16)  # [128, 128] upper-triangular keep-mask
    return cosT, sinTs, tri


def _kernel_body(tc, nc, xT, wq, wk, wv, wo, cosT, sinTs, tri, out):
    with (
        tc.tile_pool(name="cpool", bufs=1) as cpool,
        tc.tile_pool(name="qkvp", bufs=1) as qkvp,
        tc.tile_pool(name="wop", bufs=1) as wop,
        tc.tile_pool(name="dram", bufs=1, space="DRAM") as dram,
    ):
        # ---- persistent SBUF tensors ----
        tri_sb = cpool.tile([P, P], _bf, name="tri")
        ones_sb = cpool.tile([P, P], _bf, name="ones")

        qT = qkvp.tile([P, NH, T], _bf, name="qT")
        kT = qkvp.tile([P, T], _bf, name="kT")
        v_sb = [qkvp.tile([P, D], _bf, name=f"v{tt}") for tt in range(NDC)]

        wo_sb = wop.tile([P, H, 512], _bf, name="wo_sb")
        oacc = [
            [wop.tile([P, 512], _bf, name=f"oacc{mt}_{tcn}") for tcn in range(NT)]
            for mt in range(4)
        ]

        # AllGather chunk groups per head: h3 runs big->small so the last
        # (fully exposed) attention+collective+o_proj chain is the smallest
        groups = {h: ([0, 1], [2, 3]) for h in range(NH - 1)}
        groups[NH - 1] = ([2, 3], [1], [0])
        ag_in = dram.tile([NH, NT, P, 512], _bf, name="ag_in")
        ag_out = {
            (h, gi): dram.tile([KV, len(g), P, 512], _bf, name=f"ag_out{h}_{gi}")
            for h in range(NH)
            for gi, g in enumerate(groups[h])
        }
        warm_in = dram.tile([P, 8], _bf, name="warm_in")
        warm_out = dram.tile([KV, P, 8], _bf, name="warm_out")

        # ---- QKV projections (+ fused RoPE for q, k) ----
        # qT/kT layout [d, t]: out = W_chunk.T @ xT_chunk accumulated over DM.
        # Contraction (dc) is the OUTER loop with 4 t-accumulators so the PE
        # consumes x chunks as their DMAs land, and each stationary weight
        # load serves 4 matmuls.
        with (
            tc.tile_pool(name="xpool", bufs=1) as xpool,
            tc.tile_pool(name="wpool", bufs=1) as wpool,
            tc.tile_pool(name="psA", bufs=1, space="PSUM") as psA,
            tc.tile_pool(name="rope", bufs=2) as rp,
        ):
            x_sb = [
                xpool.tile([P, T], _bf, tag=f"x{dc}", name=f"x{dc}")
                for dc in range(NDC)
            ]
            wq_sb = wpool.tile([P, NDC, NH * D], _bf, name="wq_sb")
            wk_sb = wpool.tile([P, NDC, D], _bf, name="wk_sb")
            wv_sb = wpool.tile([P, NDC, D], _bf, name="wv_sb")
            cos_sb = wpool.tile([P, T], _bf, name="cos_sb")
            sin_sb = wpool.tile([P, T], _bf, name="sin_sb")
            ident = wpool.tile([P, P], _bf, name="ident")
            scr = wpool.tile([P, 8], _f32, name="scr")

            # x streams on the sync HWDGE ring; weights go on the scalar
            # (ACT) HWDGE ring so the two flows don't head-of-line block
            # each other at startup
            for dc in range(NDC):
                nc.sync.dma_start(x_sb[dc][:], xT[dc * P : (dc + 1) * P, :])
            nc.scalar.dma_start(wk_sb[:], wk)
            nc.scalar.dma_start(wv_sb[:], wv)
            nc.scalar.dma_start(cos_sb[:], cosT)
            nc.scalar.dma_start(sin_sb[:], sinTs)
            nc.scalar.dma_start(tri_sb[:], tri)
            nc.scalar.dma_start(wq_sb[:], wq)
            nc.scalar.dma_start(wo_sb[:], wo)
            nc.vector.memset(ones_sb[:], 1.0)
            masks.make_identity(nc, ident[:])

            # preload the exp activation table off the critical path, and
            # warm up the collectives path, both under the x load
            nc.scalar.activation(scr[:], ones_sb[:, :8], _EXP)
            nc.scalar.dma_start(warm_in[:], ones_sb[:, :8])
            nc.gpsimd.collective_compute(
                "AllGather",
                mybir.AluOpType.bypass,
                replica_groups=[[0, 1, 2, 3], [4, 5, 6, 7]],
                ins=[warm_in.opt()],
                outs=[warm_out.opt()],
            )

            def rope(ps, tcn, dst):
                # RoPE: rot = src*cos + swap(src)*sin_signed, all in bf16 so
                # the DVE tensor_tensor ops run in 2x mode.  The halves-swap
                # is two SBUF->SBUF DMAs on the scalar ring.
                ts = slice(tcn * 512, (tcn + 1) * 512)
                src = rp.tile([P, 512], _bf, tag="rsrc", name="rsrc")
                nc.scalar.copy(src[:], ps[:])
                swp = rp.tile([P, 512], _bf, tag="rswp", name="rswp")
                nc.scalar.dma_start(swp[0:64, :], src[64:128, :])
                nc.scalar.dma_start(swp[64:128, :], src[0:64, :])
                nc.vector.tensor_mul(src[:], src[:], cos_sb[:, ts])
                nc.vector.tensor_mul(swp[:], swp[:], sin_sb[:, ts])
                nc.vector.tensor_add(dst, src[:], swp[:])

            # Two projections share each pass (8 PSUM accumulators) so the
            # first pass consumes x chunks at least as fast as their DMAs
            # land, and each stationary weight load serves 4 matmuls.
            def wproj2(lhsA, lhsB, consumeA, consumeB):
                psa = [
                    psA.tile([P, 512], _f32, tag=f"TA{t}", name=f"TA{t}")
                    for t in range(NT)
                ]
                psb = [
                    psA.tile([P, 512], _f32, tag=f"TB{t}", name=f"TB{t}")
                    for t in range(NT)
                ]
                for dc in range(NDC):
                    for lhs, pss in ((lhsA(dc), psa), (lhsB(dc), psb)):
                        for tcn in range(NT):
                            nc.tensor.matmul(
                                pss[tcn][:],
                                lhs,
                                x_sb[dc][:, tcn * 512 : (tcn + 1) * 512],
                                start=(dc == 0),
                                stop=(dc == NDC - 1),
                            )
                for tcn in range(NT):
                    consumeA(tcn, psa[tcn])
                for tcn in range(NT):
                    consumeB(tcn, psb[tcn])

            # v: v^T = Wv^T @ x^T, then PE-transpose [128,128] blocks into the
            # natural [t, d] tiles AV needs.  The transpose PSUM slots reuse
            # the TA tags (already drained by the k ropes).
            def consume_v(tcn, ps):
                vT_sb = rp.tile([P, 512], _bf, tag="vTsb", name="vTsb")
                nc.scalar.copy(vT_sb[:], ps[:])
                for sub in range(4):
                    tt = tcn * 4 + sub
                    ptr = psA.tile([P, P], _bf, tag=f"TA{sub}", name="vtr")
                    nc.tensor.transpose(
                        ptr[:], vT_sb[:, sub * P : (sub + 1) * P], ident[:]
                    )
                    nc.vector.tensor_copy(v_sb[tt][:], ptr[:])

            # k+v first, then q pairs, so attention can begin as soon as
            # q heads land
            wproj2(
                lambda dc: wk_sb[:, dc, :],
                lambda dc: wv_sb[:, dc, :],
                lambda tcn, ps: rope(ps, tcn, kT[:, tcn * 512 : (tcn + 1) * 512]),
                consume_v,
            )
            for j0 in (0, 2):
                wproj2(
                    lambda dc, j0=j0: wq_sb[:, dc, j0 * P : (j0 + 1) * P],
                    lambda dc, j0=j0: wq_sb[:, dc, (j0 + 1) * P : (j0 + 2) * P],
                    lambda tcn, ps, j0=j0: rope(
                        ps, tcn, qT[:, j0, tcn * 512 : (tcn + 1) * 512]
                    ),
                    lambda tcn, ps, j0=j0: rope(
                        ps, tcn, qT[:, j0 + 1, tcn * 512 : (tcn + 1) * 512]
                    ),
                )

        # ---- attention + chunked AllGather + pipelined o_proj ----
        # scores^T [k, q] per head with causal block skip; the two q-chunks of
        # a group share a wide [128,1024] PSUM tile so exp and the running
        # denominator sum are single wide ops per kt.
        with (
            tc.tile_pool(name="psS", bufs=2, space="PSUM") as psS,
            tc.tile_pool(name="psAV", bufs=1, space="PSUM") as psAV,
            tc.tile_pool(name="psO", bufs=1, space="PSUM") as psO,
            tc.tile_pool(name="att", bufs=3) as ap_,
            tc.tile_pool(name="expp", bufs=4) as expp,
            tc.tile_pool(name="accp", bufs=2) as accp,
            tc.tile_pool(name="agsb", bufs=2) as agsb,
        ):
            def issue_load(ph, pgi, pg):
                # one coalesced DMA for the whole AllGather output of a chunk
                t_ = agsb.tile(
                    [P, KV, len(pg), 512], _bf, tag="agbig", name=f"ag{ph}{pgi}"
                )
                nc.sync.dma_start(t_[:], ag_out[ph, pgi].rearrange("i l p f -> p i l f"))
                return t_

            def consume_mms(ph, pg, sb):
                # o_proj contribution of global heads {4i+ph} for this chunk's
                # t-columns, accumulated into SBUF (bf16).  mt rounds ping-pong
                # the two psO banks so the PE doesn't wait on the DVE drain.
                for l, qc in enumerate(pg):
                    for mt in range(4):
                        pos = psO.tile(
                            [P, 512], _f32, tag=f"pos{mt % 2}", name=f"pos{qc}{mt}"
                        )
                        for i in range(KV):
                            nc.tensor.matmul(
                                pos[:],
                                wo_sb[:, 4 * i + ph, mt * P : (mt + 1) * P],
                                sb[:, i, l, :],
                                start=(i == 0),
                                stop=(i == KV - 1),
                            )
                        if ph == 0:
                            nc.vector.tensor_copy(oacc[mt][qc][:], pos[:])
                        else:
                            nc.vector.tensor_add(
                                oacc[mt][qc][:], oacc[mt][qc][:], pos[:]
                            )
                        if ph == NH - 1:
                            nc.sync.dma_start(
                                out[
                                    mt * P : (mt + 1) * P,
                                    qc * 512 : (qc + 1) * 512,
                                ],
                                oacc[mt][qc][:],
                            )

            pending = None
            pend_sb = None
            for h in range(NH):
                for gi, g in enumerate(groups[h]):
                    ng = len(g)
                    acc = accp.tile(
                        [P, ng * 512], _bf, tag="acc", name=f"acc{h}{gi}"
                    )
                    avs = {
                        qc: psAV.tile(
                            [P, 512], _f32, tag=f"av{i}", name=f"av{h}_{qc}"
                        )
                        for i, qc in enumerate(g)
                    }
                    def finish(qc, h=h):
                        # denominator (k-partition sum + broadcast via ones
                        # matmul), fast DVE reciprocal, normalize, ship out.
                        # Fired as soon as this q-chunk's last AV is issued so
                        # the chain hides under the remaining kt iterations.
                        ai = qc - g[0]
                        dps = psS.tile([P, 1024], _f32, tag="s", name="sden")
                        nc.tensor.matmul(
                            dps[:, :512],
                            ones_sb[:],
                            acc[:, ai * 512 : (ai + 1) * 512],
                            start=True,
                            stop=True,
                        )
                        rec = ap_.tile([P, 512], _f32, tag="rec", name="rec")
                        nc.vector.reciprocal_approx_fast(rec[:], dps[:, :512])
                        oq = ap_.tile([P, 512], _bf, tag="oq", name="oq")
                        nc.vector.tensor_mul(oq[:], avs[qc][:], rec[:])
                        nc.sync.dma_start(ag_in[h, qc][:, :], oq[:])

                    pend_av = None
                    for kt in range(4 * max(g) + 4):
                        lhs_k = kT[:, kt * P : (kt + 1) * P]
                        valid = [qc for qc in g if kt <= 4 * qc + 3]
                        ps = psS.tile([P, 1024], _f32, tag="s", name="s")
                        ex = expp.tile([P, 1024], _bf, tag="exp", name="ex")
                        exs = {}
                        for qc in valid:
                            ai = qc - g[0]
                            bound = kt // 4 == qc
                            off = 128 * (kt % 4) if bound else 0
                            lo = ai * 512 + off
                            hi = (ai + 1) * 512
                            qs = slice(qc * 512 + off, (qc + 1) * 512)
                            nc.tensor.matmul(
                                ps[:, lo:hi], lhs_k, qT[:, h, qs], start=True, stop=True
                            )
                            exs[qc] = (ex, lo, hi, off, bound)
                        lo0 = min(e[1] for e in exs.values())
                        hi0 = max(e[2] for e in exs.values())
                        nc.scalar.activation(
                            ex[:, lo0:hi0], ps[:, lo0:hi0], _EXP, scale=SCALE
                        )
                        for qc, (exw, lo, hi, off, bound) in exs.items():
                            if bound:
                                nc.vector.tensor_mul(
                                    exw[:, lo : lo + P], exw[:, lo : lo + P], tri_sb[:]
                                )
                        if kt == 0:
                            nc.vector.tensor_copy(acc[:], ex[:, : ng * 512])
                        else:
                            nc.vector.tensor_add(
                                acc[:, lo0:hi0], acc[:, lo0:hi0], ex[:, lo0:hi0]
                            )
                        # AV runs one kt behind scores so the PE never waits
                        # on ScalarE's exp round-trip
                        if pend_av is not None:
                            pkt, pexs = pend_av
                            for qc, (exw, lo, hi, off, bound) in pexs.items():
                                nc.tensor.matmul(
                                    avs[qc][:, off:],
                                    v_sb[pkt][:],
                                    exw[:, lo:hi],
                                    start=(pkt == 0),
                                    stop=(pkt == 4 * qc + 3),
                                )
                            for qc in pexs:
                                if pkt == 4 * qc + 3:
                                    finish(qc)
                        pend_av = (kt, exs)
                    pkt, pexs = pend_av
                    for qc, (exw, lo, hi, off, bound) in pexs.items():
                        nc.tensor.matmul(
                            avs[qc][:, off:],
                            v_sb[pkt][:],
                            exw[:, lo:hi],
                            start=(pkt == 0),
                            stop=(pkt == 4 * qc + 3),
                        )
                    for qc in pexs:
                        if pkt == 4 * qc + 3:
                            finish(qc)

                    # start the PREVIOUS chunk's AllGather-output load AFTER
                    # this chunk's oq writes are queued (no head-of-line
                    # blocking of the collective input path)
                    if pending is not None:
                        pend_sb = issue_load(*pending)

                    # AllGather this chunk across the 4-rank batch group
                    nc.gpsimd.collective_compute(
                        "AllGather",
                        mybir.AluOpType.bypass,
                        replica_groups=[[0, 1, 2, 3], [4, 5, 6, 7]],
                        ins=[ag_in[h, g[0] : g[0] + ng].opt()],
                        outs=[ag_out[h, gi].opt()],
                    )
                    # consume the PREVIOUS chunk's o_proj now that its
                    # AllGather has completed and its load is in flight
                    if pending is not None:
                        consume_mms(pending[0], pending[2], pend_sb)
                    pending = (h, gi, g)
            pend_sb = issue_load(*pending)
            consume_mms(pending[0], pending[2], pend_sb)


def build_nc():
    nc = bacc.Bacc(
        "TRN2", target_bir_lowering=False, debug=False, num_devices=NCORES
    )
    xT = nc.dram_tensor("xT", [DM, T], _bf, kind="ExternalInput").ap()
    wq = nc.dram_tensor("wq", [P, NDC, NH * D], _bf, kind="ExternalInput").ap()
    wk = nc.dram_tensor("wk", [P, NDC, D], _bf, kind="ExternalInput").ap()
    wv = nc.dram_tensor("wv", [P, NDC, D], _bf, kind="ExternalInput").ap()
    wo = nc.dram_tensor("wo", [P, H, 512], _bf, kind="ExternalInput").ap()
    cosT = nc.dram_tensor("cosT", [D, T], _bf, kind="ExternalInput").ap()
    sinTs = nc.dram_tensor("sinTs", [D, T], _bf, kind="ExternalInput").ap()
    tri = nc.dram_tensor("tri", [P, P], _bf, kind="ExternalInput").ap()
    out = nc.dram_tensor("out", [512, T], _bf, kind="ExternalOutput").ap()
    with tile.TileContext(nc) as tc:
        _kernel_body(tc, nc, xT, wq, wk, wv, wo, cosT, sinTs, tri, out)
    nc.finalize()
    return nc


def _chunked(w, cols):
    # [DM, cols] -> [P, NDC, cols] contiguous so the device DMA is one
    # clean 128-partition transfer
    return np.ascontiguousarray(
        w.reshape(NDC, P, cols).transpose(1, 0, 2)
    ).astype(BF16)


def make_in_maps(x, Wq, Wk, Wv, Wo):
    cosT, sinTs, tri = _host_tables()
    xTb = [np.ascontiguousarray(x[b].T).astype(BF16) for b in range(B)]
    wg = []
    for g in range(KV):
        wg.append(
            {
                "wq": _chunked(Wq[:, g * NH * D : (g + 1) * NH * D], NH * D),
                "wk": _chunked(Wk[:, g * D : (g + 1) * D], D),
                "wv": _chunked(Wv[:, g * D : (g + 1) * D], D),
                "wo": np.ascontiguousarray(
                    Wo[:, g * 512 : (g + 1) * 512].reshape(H, P, 512).transpose(1, 0, 2)
                ).astype(BF16),
            }
        )
    in_maps = []
    for r in range(NCORES):
        b, g = divmod(r, KV)
        m = {"xT": xTb[b], "cosT": cosT, "sinTs": sinTs, "tri": tri}
        m.update(wg[g])
        in_maps.append(m)
    return in_maps


def assemble(results):
    out = np.empty((B, T, DM), np.float32)
    for r in range(NCORES):
        b, g = divmod(r, KV)
        out[b, :, g * 512 : (g + 1) * 512] = results[r]["out"].T.astype(np.float32)
    return out


_NC_CACHE = {}


def get_nc():
    if "nc" not in _NC_CACHE:
        _NC_CACHE["nc"] = build_nc()
    return _NC_CACHE["nc"]


def run(x, Wq, Wk, Wv, Wo, trace=False, taps=False):
    nc = get_nc()
    in_maps = make_in_maps(x, Wq, Wk, Wv, Wo)
    res = run_bass_kernel_spmd(
        nc, in_maps, core_ids=list(range(NCORES)), trace=trace
    )
    return assemble(res.results), res


def kernel(x, Wq, Wk, Wv, Wo, mask=None, **_unused):
    x = np.asarray(x, dtype=np.float32)
    Wq = np.asarray(Wq, dtype=np.float32)
    Wk = np.asarray(Wk, dtype=np.float32)
    Wv = np.asarray(Wv, dtype=np.float32)
    Wo = np.asarray(Wo, dtype=np.float32)
    out, _ = run(x, Wq, Wk, Wv, Wo, trace=False)
    return out


# revision 24
# speedup vs baseline: 1.1940x; 1.0769x over previous
# GQA attention (B=2, T=2048, DM=2048, H=16, KV=4, D=128) + RoPE + causal mask
# on 8 TRN2 NeuronCores.
#
# Sharding: rank r = (batch b = r//4, kv-group g = r%4).  Each rank computes
# q-heads 4g..4g+3 and kv-head g for batch b (full sequence), does the SDPA
# head-sharded, then AllGathers attention outputs within each 4-rank batch
# group (chunked, overlapped with attention of later heads).  The o_proj is
# column-sharded: each rank multiplies the gathered O^T by its 512-column
# slice of Wo, accumulating in SBUF as AllGather chunks arrive, and returns
# o^T [512, 2048] bf16; the host transposes/casts and concatenates.
#
# All matmuls run in bf16 (f32 PSUM accumulation); softmax runs without
# max-subtraction (scores are bounded ~|6|); exp on ScalarE over wide
# [128,1024] PSUM tiles; denominators via bf16 running sums + a ones-[128x128]
# matmul that reduces over the k-partition axis and broadcasts; reciprocal as
# exp(-ln(x)) so ScalarE stays on one activation-table set the whole kernel.

import os
import sys

import numpy as np

for _p in ("/opt/trn_rl_repo", "/root/.axon_site/_ro/trn_rl_repo"):
    if os.path.isdir(_p) and _p not in sys.path:
        sys.path.insert(0, _p)

import ml_dtypes

import concourse.bass as bass
import concourse.mybir as mybir
import concourse.tile as tile
import concourse.masks as masks
from concourse import bacc
from concourse.bass_utils import run_bass_kernel_spmd

BF16 = ml_dtypes.bfloat16

B, T, DM = 2, 2048, 2048
H, KV, D = 16, 4, 128
NH = H // KV  # 4 local q heads per rank
P = 128
NCORES = 8
NT = T // 512  # 4 free-dim chunks of 512
NDC = DM // P  # 16 contraction chunks
SCALE = float(D) ** -0.5
ROPE_BASE = 10000.0

_bf = mybir.dt.bfloat16
_f32 = mybir.dt.float32
_EXP = mybir.ActivationFunctionType.Exp



def _host_tables():
    inv = 1.0 / (ROPE_BASE ** (np.arange(0, D, 2, dtype=np.float32) / D))
    t = np.arange(T, dtype=np.float32)
    fr = np.outer(t, inv)  # [T, 64]
    emb = np.concatenate([fr, fr], axis=-1)  # [T, D]
    cosT = np.ascontiguousarray(np.cos(emb).T).astype(BF16)  # [D, T]
    sinT = np.sin(emb).T
    sinTs = np.concatenate([-sinT[:64], sinT[64:]], axis=0)
    sinTs = np.ascontiguousarray(sinTs).astype(BF16)
    i = np.arange(P)[:, None]
    j = np.arange(P)[None, :]
    tri = (i <= j).astype(BF16)  # [128, 128] upper-triangular keep-mask
    return cosT, sinTs, tri


def _kernel_body(tc, nc, xT, wq, wk, wv, wo, cosT, sinTs, tri, out):
    with (
        tc.tile_pool(name="cpool", bufs=1) as cpool,
        tc.tile_pool(name="qkvp", bufs=1) as qkvp,
        tc.tile_pool(name="wop", bufs=1) as wop,
        tc.tile_pool(name="dram", bufs=1, space="DRAM") as dram,
    ):
        # ---- persistent SBUF tensors ----
        tri_sb = cpool.tile([P, P], _bf, name="tri")
        ones_sb = cpool.tile([P, P], _bf, name="ones")

        qT = qkvp.tile([P, NH, T], _bf, name="qT")
        kT = qkvp.tile([P, T], _bf, name="kT")
        v_sb = [qkvp.tile([P, D], _bf, name=f"v{tt}") for tt in range(NDC)]

        wo_sb = wop.tile([P, H, 512], _bf, name="wo_sb")
        oacc = [
            [wop.tile([P, 512], _bf, name=f"oacc{mt}_{tcn}") for tcn in range(NT)]
            for mt in range(4)
        ]

        # AllGather chunk groups per head: h3 runs big->small so the last
        # (fully exposed) attention+collective+o_proj chain is the smallest
        groups = {h: ([0, 1], [2, 3]) for h in range(NH - 1)}
        groups[NH - 1] = ([2, 3], [1], [0])
        ag_in = dram.tile([NH, NT, P, 512], _bf, name="ag_in")
        ag_out = {
            (h, gi): dram.tile([KV, len(g), P, 512], _bf, name=f"ag_out{h}_{gi}")
            for h in range(NH)
            for gi, g in enumerate(groups[h])
        }
        warm_in = dram.tile([P, 8], _bf, name="warm_in")
        warm_out = dram.tile([KV, P, 8], _bf, name="warm_out")

        # ---- QKV projections (+ fused RoPE for q, k) ----
        # qT/kT layout [d, t]: out = W_chunk.T @ xT_chunk accumulated over DM.
        # Contraction (dc) is the OUTER loop with 4 t-accumulators so the PE
        # consumes x chunks as their DMAs land, and each stationary weight
        # load serves 4 matmuls.
        with (
            tc.tile_pool(name="xpool", bufs=1) as xpool,
            tc.tile_pool(name="wpool", bufs=1) as wpool,
            tc.tile_pool(name="psA", bufs=1, space="PSUM") as psA,
            tc.tile_pool(name="rope", bufs=2) as rp,
        ):
            x_sb = [
                xpool.tile([P, T], _bf, tag=f"x{dc}", name=f"x{dc}")
                for dc in range(NDC)
            ]
            wq_sb = wpool.tile([P, NDC, NH * D], _bf, name="wq_sb")
            wk_sb = wpool.tile([P, NDC, D], _bf, name="wk_sb")
            wv_sb = wpool.tile([P, NDC, D], _bf, name="wv_sb")
            cos_sb = wpool.tile([P, T], _bf, name="cos_sb")
            sin_sb = wpool.tile([P, T], _bf, name="sin_sb")
            ident = wpool.tile([P, P], _bf, name="ident")
            scr = wpool.tile([P, 8], _f32, name="scr")

            # x streams on the sync HWDGE ring; weights go on the scalar
            # (ACT) HWDGE ring so the two flows don't head-of-line block
            # each other at startup
            for dc in range(NDC):
                nc.sync.dma_start(x_sb[dc][:], xT[dc * P : (dc + 1) * P, :])
            nc.scalar.dma_start(wk_sb[:], wk)
            nc.scalar.dma_start(wv_sb[:], wv)
            nc.scalar.dma_start(cos_sb[:], cosT)
            nc.scalar.dma_start(sin_sb[:], sinTs)
            nc.scalar.dma_start(tri_sb[:], tri)
            nc.scalar.dma_start(wq_sb[:], wq)
            nc.scalar.dma_start(wo_sb[:], wo)
            nc.vector.memset(ones_sb[:], 1.0)
            masks.make_identity(nc, ident[:])

            # preload the exp activation table off the critical path, and
            # warm up the collectives path, both under the x load
            nc.scalar.activation(scr[:], ones_sb[:, :8], _EXP)
            nc.scalar.dma_start(warm_in[:], ones_sb[:, :8])
            nc.gpsimd.collective_compute(
                "AllGather",
                mybir.AluOpType.bypass,
                replica_groups=[[0, 1, 2, 3], [4, 5, 6, 7]],
                ins=[warm_in.opt()],
                outs=[warm_out.opt()],
            )

            def rope(ps, tcn, dst):
                # RoPE: rot = src*cos + swap(src)*sin_signed, all in bf16 so
                # the DVE tensor_tensor ops run in 2x mode.  The halves-swap
                # is two SBUF->SBUF DMAs on the scalar ring.
                ts = slice(tcn * 512, (tcn + 1) * 512)
                src = rp.tile([P, 512], _bf, tag="rsrc", name="rsrc")
                nc.scalar.copy(src[:], ps[:])
                swp = rp.tile([P, 512], _bf, tag="rswp", name="rswp")
                nc.sync.dma_start(swp[0:64, :], src[64:128, :])
                nc.sync.dma_start(swp[64:128, :], src[0:64, :])
                nc.vector.tensor_mul(src[:], src[:], cos_sb[:, ts])
                nc.vector.tensor_mul(swp[:], swp[:], sin_sb[:, ts])
                nc.vector.tensor_add(dst, src[:], swp[:])

            # Two projections share each pass (8 PSUM accumulators) so the
            # first pass consumes x chunks at least as fast as their DMAs
            # land, and each stationary weight load serves 4 matmuls.
            def wproj2(lhsA, lhsB, consumeA, consumeB):
                psa = [
                    psA.tile([P, 512], _f32, tag=f"TA{t}", name=f"TA{t}")
                    for t in range(NT)
                ]
                psb = [
                    psA.tile([P, 512], _f32, tag=f"TB{t}", name=f"TB{t}")
                    for t in range(NT)
                ]
                for dc in range(NDC):
                    for lhs, pss in ((lhsA(dc), psa), (lhsB(dc), psb)):
                        for tcn in range(NT):
                            nc.tensor.matmul(
                                pss[tcn][:],
                                lhs,
                                x_sb[dc][:, tcn * 512 : (tcn + 1) * 512],
                                start=(dc == 0),
                                stop=(dc == NDC - 1),
                            )
                for tcn in range(NT):
                    consumeA(tcn, psa[tcn])
                for tcn in range(NT):
                    consumeB(tcn, psb[tcn])

            # v: v^T = Wv^T @ x^T, then PE-transpose [128,128] blocks into the
            # natural [t, d] tiles AV needs.  The transpose PSUM slots reuse
            # the TA tags (already drained by the k ropes).
            def consume_v(tcn, ps):
                vT_sb = rp.tile([P, 512], _bf, tag="vTsb", name="vTsb")
                nc.scalar.copy(vT_sb[:], ps[:])
                for sub in range(4):
                    tt = tcn * 4 + sub
                    ptr = psA.tile([P, P], _bf, tag=f"TA{sub}", name="vtr")
                    nc.tensor.transpose(
                        ptr[:], vT_sb[:, sub * P : (sub + 1) * P], ident[:]
                    )
                    nc.vector.tensor_copy(v_sb[tt][:], ptr[:])

            # Single-projection pass (4 accumulators on one tag group): a
            # head's rope chains then overlap the NEXT head's matmul pass.
            def wproj1(lhs_of_dc, consume, tg):
                pss = [
                    psA.tile([P, 512], _f32, tag=f"{tg}{t}", name=f"{tg}{t}")
                    for t in range(NT)
                ]
                for dc in range(NDC):
                    lhs = lhs_of_dc(dc)
                    for tcn in range(NT):
                        nc.tensor.matmul(
                            pss[tcn][:],
                            lhs,
                            x_sb[dc][:, tcn * 512 : (tcn + 1) * 512],
                            start=(dc == 0),
                            stop=(dc == NDC - 1),
                        )
                for tcn in range(NT):
                    consume(tcn, pss[tcn])

            # k+v share the first pass (dense PE while x streams in); the q
            # heads run as single passes on alternating tag groups so only
            # the last head's ropes are exposed at the phase boundary
            wproj2(
                lambda dc: wk_sb[:, dc, :],
                lambda dc: wv_sb[:, dc, :],
                lambda tcn, ps: rope(ps, tcn, kT[:, tcn * 512 : (tcn + 1) * 512]),
                consume_v,
            )
            for j in range(NH):
                wproj1(
                    lambda dc, j=j: wq_sb[:, dc, j * P : (j + 1) * P],
                    lambda tcn, ps, j=j: rope(
                        ps, tcn, qT[:, j, tcn * 512 : (tcn + 1) * 512]
                    ),
                    "TA" if j % 2 == 0 else "TB",
                )

        # ---- attention + chunked AllGather + pipelined o_proj ----
        # scores^T [k, q] per head with causal block skip; the two q-chunks of
        # a group share a wide [128,1024] PSUM tile so exp and the running
        # denominator sum are single wide ops per kt.
        with (
            tc.tile_pool(name="psS", bufs=2, space="PSUM") as psS,
            tc.tile_pool(name="psAV", bufs=1, space="PSUM") as psAV,
            tc.tile_pool(name="psO", bufs=1, space="PSUM") as psO,
            tc.tile_pool(name="att", bufs=3) as ap_,
            tc.tile_pool(name="expp", bufs=4) as expp,
            tc.tile_pool(name="accp", bufs=2) as accp,
            tc.tile_pool(name="agsb", bufs=3) as agsb,
        ):
            def issue_load(ph, pgi, pg):
                # one coalesced DMA for the whole AllGather output of a chunk
                t_ = agsb.tile(
                    [P, KV, len(pg), 512], _bf, tag="agbig", name=f"ag{ph}{pgi}"
                )
                nc.sync.dma_start(t_[:], ag_out[ph, pgi].rearrange("i l p f -> p i l f"))
                return t_

            def consume_mms(ph, pg, sb, qcs=None):
                # o_proj contribution of global heads {4i+ph} for this chunk's
                # t-columns, accumulated into SBUF (bf16).  mt rounds ping-pong
                # the two psO banks so the PE doesn't wait on the DVE drain.
                for l, qc in enumerate(pg):
                    if qcs is not None and qc not in qcs:
                        continue
                    for mt in range(4):
                        pos = psO.tile(
                            [P, 512], _f32, tag=f"pos{mt % 2}", name=f"pos{qc}{mt}"
                        )
                        for i in range(KV):
                            nc.tensor.matmul(
                                pos[:],
                                wo_sb[:, 4 * i + ph, mt * P : (mt + 1) * P],
                                sb[:, i, l, :],
                                start=(i == 0),
                                stop=(i == KV - 1),
                            )
                        if ph == 0:
                            nc.vector.tensor_copy(oacc[mt][qc][:], pos[:])
                        else:
                            nc.vector.tensor_add(
                                oacc[mt][qc][:], oacc[mt][qc][:], pos[:]
                            )
                        if ph == NH - 1:
                            nc.sync.dma_start(
                                out[
                                    mt * P : (mt + 1) * P,
                                    qc * 512 : (qc + 1) * 512,
                                ],
                                oacc[mt][qc][:],
                            )

            pending = None
            pend_sb = None
            for h in range(NH):
                for gi, g in enumerate(groups[h]):
                    ng = len(g)
                    acc = accp.tile(
                        [P, ng * 512], _bf, tag="acc", name=f"acc{h}{gi}"
                    )
                    avs = {
                        qc: psAV.tile(
                            [P, 512], _f32, tag=f"av{i}", name=f"av{h}_{qc}"
                        )
                        for i, qc in enumerate(g)
                    }
                    def finish(qc, h=h):
                        # denominator (k-partition sum + broadcast via ones
                        # matmul), fast DVE reciprocal, normalize, ship out.
                        # Fired as soon as this q-chunk's last AV is issued so
                        # the chain hides under the remaining kt iterations.
                        ai = qc - g[0]
                        dps = psS.tile([P, 1024], _f32, tag="s", name="sden")
                        nc.tensor.matmul(
                            dps[:, :512],
                            ones_sb[:],
                            acc[:, ai * 512 : (ai + 1) * 512],
                            start=True,
                            stop=True,
                        )
                        rec = ap_.tile([P, 512], _f32, tag="rec", name="rec")
                        nc.vector.reciprocal_approx_fast(rec[:], dps[:, :512])
                        oq = ap_.tile([P, 512], _bf, tag="oq", name="oq")
                        nc.vector.tensor_mul(oq[:], avs[qc][:], rec[:])
                        # scalar ring: the sync ring carries the (long-waiting)
                        # AllGather-output loads, which must not delay this
                        # write -> trigger path
                        nc.scalar.dma_start(ag_in[h, qc][:, :], oq[:])

                    pend_av = None
                    for kt in range(4 * max(g) + 4):
                        lhs_k = kT[:, kt * P : (kt + 1) * P]
                        valid = [qc for qc in g if kt <= 4 * qc + 3]
                        ps = psS.tile([P, 1024], _f32, tag="s", name="s")
                        ex = expp.tile([P, 1024], _bf, tag="exp", name="ex")
                        exs = {}
                        for qc in valid:
                            ai = qc - g[0]
                            bound = kt // 4 == qc
                            off = 128 * (kt % 4) if bound else 0
                            lo = ai * 512 + off
                            hi = (ai + 1) * 512
                            qs = slice(qc * 512 + off, (qc + 1) * 512)
                            nc.tensor.matmul(
                                ps[:, lo:hi], lhs_k, qT[:, h, qs], start=True, stop=True
                            )
                            exs[qc] = (ex, lo, hi, off, bound)
                        lo0 = min(e[1] for e in exs.values())
                        hi0 = max(e[2] for e in exs.values())
                        nc.scalar.activation(
                            ex[:, lo0:hi0], ps[:, lo0:hi0], _EXP, scale=SCALE
                        )
                        for qc, (exw, lo, hi, off, bound) in exs.items():
                            if bound:
                                nc.vector.tensor_mul(
                                    exw[:, lo : lo + P], exw[:, lo : lo + P], tri_sb[:]
                                )
                        if kt == 0:
                            nc.vector.tensor_copy(acc[:], ex[:, : ng * 512])
                        else:
                            nc.vector.tensor_add(
                                acc[:, lo0:hi0], acc[:, lo0:hi0], ex[:, lo0:hi0]
                            )
                        # AV runs one kt behind scores so the PE never waits
                        # on ScalarE's exp round-trip
                        if pend_av is not None:
                            pkt, pexs = pend_av
                            for qc, (exw, lo, hi, off, bound) in pexs.items():
                                nc.tensor.matmul(
                                    avs[qc][:, off:],
                                    v_sb[pkt][:],
                                    exw[:, lo:hi],
                                    start=(pkt == 0),
                                    stop=(pkt == 4 * qc + 3),
                                )
                            for qc in pexs:
                                if pkt == 4 * qc + 3:
                                    finish(qc)
                        pend_av = (kt, exs)
                    pkt, pexs = pend_av
                    for qc, (exw, lo, hi, off, bound) in pexs.items():
                        nc.tensor.matmul(
                            avs[qc][:, off:],
                            v_sb[pkt][:],
                            exw[:, lo:hi],
                            start=(pkt == 0),
                            stop=(pkt == 4 * qc + 3),
                        )
                    for qc in pexs:
                        if pkt == 4 * qc + 3:
                            finish(qc)

                    # start the PREVIOUS chunk's AllGather-output load AFTER
                    # this chunk's oq writes are queued (no head-of-line
                    # blocking of the collective input path)
                    if pending is not None:
                        pend_sb = issue_load(*pending)

                    # AllGather this chunk across the 4-rank batch group
                    nc.gpsimd.collective_compute(
                        "AllGather",
                        mybir.AluOpType.bypass,
                        replica_groups=[[0, 1, 2, 3], [4, 5, 6, 7]],
                        ins=[ag_in[h, g[0] : g[0] + ng].opt()],
                        outs=[ag_out[h, gi].opt()],
                    )
                    # consume the PREVIOUS chunk's o_proj now that its
                    # AllGather has completed and its load is in flight.
                    # Half of h3's first chunk is deferred to the very end,
                    # where it fills the PE idle while the last AllGather
                    # lands.
                    if pending is not None:
                        ph, pgi, pg = pending
                        if (ph, pgi) == (NH - 1, 0):
                            consume_mms(ph, pg, pend_sb, qcs=[pg[0]])
                            deferred = (ph, pg, pend_sb, [pg[1]])
                        else:
                            consume_mms(ph, pg, pend_sb)
                    pending = (h, gi, g)
            pend_sb = issue_load(*pending)
            # deferred h3 work streams on the PE while the final AllGather
            # and its load complete
            consume_mms(deferred[0], deferred[1], deferred[2], qcs=deferred[3])
            consume_mms(pending[0], pending[2], pend_sb)


def build_nc():
    nc = bacc.Bacc(
        "TRN2", target_bir_lowering=False, debug=False, num_devices=NCORES
    )
    xT = nc.dram_tensor("xT", [DM, T], _bf, kind="ExternalInput").ap()
    wq = nc.dram_tensor("wq", [P, NDC, NH * D], _bf, kind="ExternalInput").ap()
    wk = nc.dram_tensor("wk", [P, NDC, D], _bf, kind="ExternalInput").ap()
    wv = nc.dram_tensor("wv", [P, NDC, D], _bf, kind="ExternalInput").ap()
    wo = nc.dram_tensor("wo", [P, H, 512], _bf, kind="ExternalInput").ap()
    cosT = nc.dram_tensor("cosT", [D, T], _bf, kind="ExternalInput").ap()
    sinTs = nc.dram_tensor("sinTs", [D, T], _bf, kind="ExternalInput").ap()
    tri = nc.dram_tensor("tri", [P, P], _bf, kind="ExternalInput").ap()
    out = nc.dram_tensor("out", [512, T], _bf, kind="ExternalOutput").ap()
    with tile.TileContext(nc) as tc:
        _kernel_body(tc, nc, xT, wq, wk, wv, wo, cosT, sinTs, tri, out)
    nc.finalize()
    return nc


def _chunked(w, cols):
    # [DM, cols] -> [P, NDC, cols] contiguous so the device DMA is one
    # clean 128-partition transfer
    return np.ascontiguousarray(
        w.reshape(NDC, P, cols).transpose(1, 0, 2)
    ).astype(BF16)


def make_in_maps(x, Wq, Wk, Wv, Wo):
    cosT, sinTs, tri = _host_tables()
    xTb = [np.ascontiguousarray(x[b].T).astype(BF16) for b in range(B)]
    wg = []
    for g in range(KV):
        wg.append(
            {
                "wq": _chunked(Wq[:, g * NH * D : (g + 1) * NH * D], NH * D),
                "wk": _chunked(Wk[:, g * D : (g + 1) * D], D),
                "wv": _chunked(Wv[:, g * D : (g + 1) * D], D),
                "wo": np.ascontiguousarray(
                    Wo[:, g * 512 : (g + 1) * 512].reshape(H, P, 512).transpose(1, 0, 2)
                ).astype(BF16),
            }
        )
    in_maps = []
    for r in range(NCORES):
        b, g = divmod(r, KV)
        m = {"xT": xTb[b], "cosT": cosT, "sinTs": sinTs, "tri": tri}
        m.update(wg[g])
        in_maps.append(m)
    return in_maps


def assemble(results):
    out = np.empty((B, T, DM), np.float32)
    for r in range(NCORES):
        b, g = divmod(r, KV)
        out[b, :, g * 512 : (g + 1) * 512] = results[r]["out"].T.astype(np.float32)
    return out


_NC_CACHE = {}


def get_nc():
    if "nc" not in _NC_CACHE:
        _NC_CACHE["nc"] = build_nc()
    return _NC_CACHE["nc"]


def run(x, Wq, Wk, Wv, Wo, trace=False, taps=False):
    nc = get_nc()
    in_maps = make_in_maps(x, Wq, Wk, Wv, Wo)
    res = run_bass_kernel_spmd(
        nc, in_maps, core_ids=list(range(NCORES)), trace=trace
    )
    return assemble(res.results), res


def kernel(x, Wq, Wk, Wv, Wo, mask=None, **_unused):
    x = np.asarray(x, dtype=np.float32)
    Wq = np.asarray(Wq, dtype=np.float32)
    Wk = np.asarray(Wk, dtype=np.float32)
    Wv = np.asarray(Wv, dtype=np.float32)
    Wo = np.asarray(Wo, dtype=np.float32)
    out, _ = run(x, Wq, Wk, Wv, Wo, trace=False)
    return out
